# revision 2
# baseline (speedup 1.0000x reference)
"""AttentiveMatchingLayer TRN2 kernel.

Math (per batch, validated against the jax reference):
  ssa[t] = sum_d a[t,d]^2 ; ssb likewise ; stok = 1/sqrt(ssa*ssb)
  as = a * stok[:,None]                     # carries BOTH l2 norms
  alpha[d,e] = sum_t b[t,d] * as[t,e]       # == ref alpha (norms folded)
  s_al[e] = 1/sqrt(sum_d alpha[d,e]^2)
  hmT[e,t] = sum_d alpha[d,e] * b[t,d]      # s_al folded into w2 scalings
     (differs from ref hmean by a per-token positive factor 1/rb[t],
      which cancels in the final cosine)
  num[t,p] = sum_d (a*hmT) (W2*s_al) ; sa = sum_d a^2 W2 ; sh = sum_d hmT^2 (W2*s_al^2)
  persp = num / sqrt((sa+eps)*(sh+eps))
Sharding: data-parallel over batch B=32 across 8 cores (4 batches/core).

Implementation notes:
- float16 streaming tensors (10-bit mantissa: ~3e-4 end-to-end error vs the
  f32 reference); f32 PSUM accumulation; f32 norms/final math.
- Phase-major over the core's 4 batches: each engine phase is one dense 4x
  burst (keeps the PE HAM-warm and hides the serial norm chains).
- Per-token / per-column sumsq land token-major on partitions by riding an
  extra ones-column of w2t (ssa), by square+reduce in [t,d] layout (ssb),
  or as rhs=ones matmuls (s_al) — no partition scatters anywhere.
"""

import numpy as np
from contextlib import ExitStack

import concourse.bacc as bacc
import concourse.bass as bass
import concourse.tile as tile
from concourse import masks, mybir

B, T, D, P = 32, 1024, 256, 20
PA = P + 1         # w2t augmented with a ones column (-> ssa)
N_CORES = 8
NB = B // N_CORES  # batches per core
TC = T // 128      # 8 token chunks
DC = D // 128      # 2 d chunks
F32 = mybir.dt.float32
F16 = mybir.dt.float16
EPS = 1e-12
Square = mybir.ActivationFunctionType.Square
Sqrt = mybir.ActivationFunctionType.Sqrt


def build_kernel():
    nc = bacc.Bacc("TRN2", target_bir_lowering=False, debug=False,
                   num_devices=N_CORES)
    a_in = nc.declare_dram_parameter("a", [NB, T, D], F32, isOutput=False)
    b_in = nc.declare_dram_parameter("b", [NB, T, D], F32, isOutput=False)
    w2t_in = nc.declare_dram_parameter("w2t", [D, PA], F32, isOutput=False)
    out_d = nc.declare_dram_parameter("out", [NB, T, P], F32, isOutput=True)

    with tile.TileContext(nc) as tc, ExitStack() as ctx:
        consts = ctx.enter_context(tc.tile_pool(name="consts", bufs=1))
        p4 = ctx.enter_context(tc.tile_pool(name="p4", bufs=NB))
        pscr = ctx.enter_context(tc.tile_pool(name="pscr", bufs=2))
        ps = ctx.enter_context(tc.tile_pool(name="ps", bufs=8, space="PSUM"))

        identf = consts.tile([128, 128], F32)
        masks.make_identity(nc, identf[:])
        ident = consts.tile([128, 128], F16)
        nc.vector.tensor_copy(ident[:], identf[:])
        ones = consts.tile([128, 1], F16)
        nc.vector.memset(ones[:], 1.0)
        eps_sb = consts.tile([128, 1], F32)
        nc.vector.memset(eps_sb[:], EPS)
        w2t = consts.tile([128, DC, PA], F16)
        nc.gpsimd.dma_start(
            out=w2t[:], in_=w2t_in.ap().rearrange("(dc p) w -> p dc w", p=128))

        NBR = range(NB)
        # ---- loads (f32 -> f16 cast in SWDGE DMA) ----
        a_sb, b_sb = [], []
        for b in NBR:
            a_sb.append(p4.tile([128, TC, D], F16, tag="a_sb", name=f"a_sb{b}"))
            nc.gpsimd.dma_start(
                out=a_sb[b][:],
                in_=a_in.ap()[b].rearrange("(p c) d -> p c d", p=128))
            b_sb.append(p4.tile([128, TC, D], F16, tag="b_sb", name=f"b_sb{b}"))
            nc.gpsimd.dma_start(
                out=b_sb[b][:],
                in_=b_in.ap()[b].rearrange("(p c) d -> p c d", p=128))

        # ---- ssb: square (GpSimd) + reduce over d (DVE), [t, d] layout ----
        ssb_sb, stok, sa_sb = [], [], []
        for b in NBR:
            sq = pscr.tile([128, TC, D], F16, tag="sq_scr", name=f"sq{b}")
            nc.gpsimd.tensor_mul(
                sq[:].rearrange("p c d -> p (c d)"),
                b_sb[b][:].rearrange("p c d -> p (c d)"),
                b_sb[b][:].rearrange("p c d -> p (c d)"))
            ssb_sb.append(p4.tile([128, TC], F32, tag="ssb", name=f"ssb{b}"))
            nc.vector.reduce_sum(ssb_sb[b][:], sq[:], axis=mybir.AxisListType.X)

        # ---- transposes (PE) + prompt evacuation ----
        aT_sb, bT_sb, asq_sb, as_sb = [], [], [], []
        for b in NBR:
            aT_ps = [ps.tile([128, 1024], F16, tag="ps", name=f"aT_ps{b}_{i}")
                     for i in range(DC)]
            bT_ps = [ps.tile([128, 1024], F16, tag="ps", name=f"bT_ps{b}_{i}")
                     for i in range(DC)]
            for dc in range(DC):
                for c in range(TC):
                    nc.tensor.transpose(
                        out=aT_ps[dc][:, c * 128:(c + 1) * 128],
                        in_=a_sb[b][:, c, dc * 128:(dc + 1) * 128],
                        identity=ident[:])
                    nc.tensor.transpose(
                        out=bT_ps[dc][:, c * 128:(c + 1) * 128],
                        in_=b_sb[b][:, c, dc * 128:(dc + 1) * 128],
                        identity=ident[:])
            aT_sb.append(p4.tile([128, DC, T], F16, tag="aT_sb", name=f"aT{b}"))
            bT_sb.append(p4.tile([128, DC, T], F16, tag="bT_sb", name=f"bT{b}"))
            asq_sb.append(p4.tile([128, DC, T], F16, tag="asq_sb", name=f"asq{b}"))
            for dc in range(DC):
                nc.scalar.copy(aT_sb[b][:, dc, :], aT_ps[dc][:])
                nc.scalar.copy(bT_sb[b][:, dc, :], bT_ps[dc][:])
                nc.scalar.activation(asq_sb[b][:, dc, :], aT_ps[dc][:], Square)

            # early sa matmul right behind this batch's transposes: keeps PE
            # dense while the stok chain of earlier batches runs on DVE/ACT
            sa_ps = [ps.tile([128, (TC // 2) * PA], F32, tag="ps",
                             name=f"sa_ps{b}_{i}") for i in range(2)]
            for c in range(TC):
                for dc in range(DC):
                    nc.tensor.matmul(
                        sa_ps[c % 2][:, (c // 2) * PA:(c // 2) * PA + PA],
                        lhsT=asq_sb[b][:, dc, c * 128:(c + 1) * 128],
                        rhs=w2t[:, dc, :],
                        start=(dc == 0), stop=(dc == DC - 1))
            sa_sb.append(p4.tile([128, 2, (TC // 2) * PA], F32, tag="sa_sb",
                                 name=f"sa_sb{b}"))
            for h in range(2):
                nc.vector.tensor_copy(sa_sb[b][:, h, :], sa_ps[h][:])

            # stok = 1/sqrt(ssa*ssb) ; as = a*stok (chains overlap next batch)
            st = p4.tile([128, TC], F32, tag="stok", name=f"stok{b}")
            stok.append(st)
            for h in range(2):
                nc.vector.tensor_mul(
                    st[:, h::2],
                    sa_sb[b][:, h, :].rearrange("q (c w) -> q c w", w=PA)[:, :, P],
                    ssb_sb[b][:, h::2])
            nc.scalar.activation(st[:], st[:], Sqrt)
            nc.vector.reciprocal(st[:], st[:])
            as_sb.append(p4.tile([128, TC, D], F16, tag="as_sb", name=f"as_sb{b}"))
            for c in range(TC):
                nc.vector.tensor_scalar_mul(
                    as_sb[b][:, c, :], a_sb[b][:, c, :], st[:, c:c + 1])

        # ---- alpha[d, e] = sum_t b[t,d] as[t,e] (PE, bank ping-pong) ----
        alpha_sb, alsq_sb = [], []
        for b in NBR:
            alpha_ps = [ps.tile([128, 256], F32, tag="ps", name=f"al_ps{b}_{i}")
                        for i in range(DC)]
            for c in range(TC):
                for dc in range(DC):
                    nc.tensor.matmul(
                        alpha_ps[dc][:],
                        lhsT=b_sb[b][:, c, dc * 128:(dc + 1) * 128],
                        rhs=as_sb[b][:, c, :],
                        start=(c == 0), stop=(c == TC - 1))
            alpha_sb.append(p4.tile([128, DC, 256], F16, tag="alpha_sb",
                                    name=f"alpha_sb{b}"))
            alsq_sb.append(p4.tile([128, DC, 256], F16, tag="alsq_sb",
                                   name=f"alsq_sb{b}"))
            for dc in range(DC):
                nc.vector.tensor_copy(alpha_sb[b][:, dc, :], alpha_ps[dc][:])
                nc.scalar.activation(alsq_sb[b][:, dc, :], alpha_ps[dc][:], Square)

        # ---- s_al + folded w2 scalings ----
        w2sal, w2sal2 = [], []
        for b in NBR:
            sal_ps = ps.tile([128, 2], F32, tag="ps", name=f"sal_ps{b}")
            for ec in range(2):
                for dc in range(DC):
                    nc.tensor.matmul(
                        sal_ps[:, ec:ec + 1],
                        lhsT=alsq_sb[b][:, dc, ec * 128:(ec + 1) * 128],
                        rhs=ones[:],
                        start=(dc == 0), stop=(dc == DC - 1))
            sal = p4.tile([128, 2], F32, tag="sal", name=f"sal{b}")
            nc.vector.tensor_copy(sal[:], sal_ps[:])
            nc.scalar.activation(sal[:], sal[:], Sqrt)
            nc.vector.reciprocal(sal[:], sal[:])
            ws = p4.tile([128, DC, P], F16, tag="w2sal", name=f"w2sal{b}")
            ws2 = p4.tile([128, DC, P], F16, tag="w2sal2", name=f"w2sal2{b}")
            for dc in range(DC):
                nc.vector.tensor_scalar_mul(
                    ws[:, dc, :], w2t[:, dc, 0:P], sal[:, dc:dc + 1])
                nc.vector.tensor_scalar_mul(
                    ws2[:, dc, :], ws[:, dc, :], sal[:, dc:dc + 1])
            w2sal.append(ws)
            w2sal2.append(ws2)

        # ---- hmT (PE) ; prod = aT*hmT (DVE) ; hmsq = hmT^2 (ACT) ----
        prod_sb, hmsq_sb = [], []
        for b in NBR:
            prod_sb.append(p4.tile([128, 2, T], F16, tag="prod_sb",
                                   name=f"prod{b}"))
            hmsq_sb.append(p4.tile([128, 2, T], F16, tag="hmsq_sb",
                                   name=f"hmsq{b}"))
        for b in NBR:
            for ec in range(2):
                for t2 in range(2):
                    hp = ps.tile([128, 512], F32, tag="ps",
                                 name=f"hm_ps{b}_{ec}_{t2}")
                    for dc in range(DC):
                        nc.tensor.matmul(
                            hp[:],
                            lhsT=alpha_sb[b][:, dc, ec * 128:(ec + 1) * 128],
                            rhs=bT_sb[b][:, dc, t2 * 512:(t2 + 1) * 512],
                            start=(dc == 0), stop=(dc == DC - 1))
                    sl = slice(t2 * 512, t2 * 512 + 512)
                    nc.vector.tensor_mul(
                        prod_sb[b][:, ec, sl], aT_sb[b][:, ec, sl], hp[:])
                    nc.scalar.activation(hmsq_sb[b][:, ec, sl], hp[:], Square)

        # ---- finals in [t, p] layout (PE, bank ping-pong) + division ----
        for b in NBR:
            fin_ps = [ps.tile([128, (TC // 2) * P], F32, tag="ps",
                              name=f"fin_ps{b}_{i}") for i in range(4)]
            for q, (src, rhs) in enumerate(
                    ((prod_sb[b], w2sal[b]), (hmsq_sb[b], w2sal2[b]))):
                for c in range(TC):
                    fp = fin_ps[q * 2 + c % 2]
                    for dc in range(DC):
                        nc.tensor.matmul(
                            fp[:, (c // 2) * P:(c // 2) * P + P],
                            lhsT=src[:, dc, c * 128:(c + 1) * 128],
                            rhs=rhs[:, dc, :],
                            start=(dc == 0), stop=(dc == DC - 1))
            # persp = num / sqrt((sa+eps)(sh+eps)); h-major (c = 2j+h)
            ssh = pscr.tile([128, 2, 4 * P], F32, tag="ssh", name=f"ssh{b}")
            pnum = pscr.tile([128, 2, 4 * P], F32, tag="pnum", name=f"pnum{b}")
            den = pscr.tile([128, 2, 4 * P], F32, tag="den", name=f"den{b}")
            for h in range(2):
                nc.vector.tensor_copy(ssh[:, h, :], fin_ps[2 + h][:])
                nc.scalar.copy(pnum[:, h, :], fin_ps[h][:])
                nc.vector.tensor_mul(
                    den[:, h, :].rearrange("q (j w) -> q j w", w=P),
                    sa_sb[b][:, h, :].rearrange("q (j w) -> q j w", w=PA)[:, :, 0:P],
                    ssh[:, h, :].rearrange("q (j w) -> q j w", w=P))
            nc.scalar.activation(
                den[:].rearrange("q h jw -> q (h jw)"),
                den[:].rearrange("q h jw -> q (h jw)"), Sqrt, bias=eps_sb[:])
            nc.vector.reciprocal(
                den[:].rearrange("q h jw -> q (h jw)"),
                den[:].rearrange("q h jw -> q (h jw)"))
            persp = pscr.tile([128, 2, 4 * P], F32, tag="persp", name=f"persp{b}")
            for h in range(2):
                nc.vector.tensor_mul(persp[:, h, :], pnum[:, h, :], den[:, h, :])
                nc.sync.dma_start(
                    out=out_d.ap()[b].rearrange(
                        "(q j h) w -> q h j w", h=2, j=4)[:, h],
                    in_=persp[:, h, :].rearrange("q (j w) -> q j w", w=P))

    nc.compile()
    return nc


_NC_CACHE = None


def _get_nc():
    global _NC_CACHE
    if _NC_CACHE is None:
        _NC_CACHE = build_kernel()
    return _NC_CACHE


def _make_in_maps(inp_a, inp_b, W):
    inp_a = np.ascontiguousarray(np.asarray(inp_a, dtype=np.float32))
    inp_b = np.ascontiguousarray(np.asarray(inp_b, dtype=np.float32))
    W = np.asarray(W, dtype=np.float32)
    w2t = np.ones((D, PA), dtype=np.float32)
    w2t[:, :P] = (W * W).T
    return [
        {"a": inp_a[k * NB:(k + 1) * NB], "b": inp_b[k * NB:(k + 1) * NB],
         "w2t": w2t}
        for k in range(N_CORES)
    ]


def kernel(inp_a, inp_b, W):
    from concourse.bass_utils import run_bass_kernel_spmd
    nc = _get_nc()
    in_maps = _make_in_maps(inp_a, inp_b, W)
    res = run_bass_kernel_spmd(nc, in_maps, list(range(N_CORES)))
    persp = np.concatenate(
        [res.results[k]["out"] for k in range(N_CORES)], axis=0)
    return (persp, persp)


if __name__ == "__main__":
    rng = np.random.default_rng(0)
    inputs = {
        "inp_a": rng.standard_normal((B, T, D), dtype=np.float32),
        "inp_b": rng.standard_normal((B, T, D), dtype=np.float32),
        "W": rng.uniform(-0.05, 0.05, (P, D)).astype(np.float32),
    }
    out = kernel(**inputs)
    print("ok", out[0].shape, out[0].dtype)

